# revision 25
# baseline (speedup 1.0000x reference)
"""Multi-head self-attention (B=4, S=2048, D=1024, H=16, Hd=64) on 8 TRN2 cores.

Sharding: core c -> (batch b = c//2, head-group g = c%2 of 8 heads).
Each core computes its batch's 8 heads end-to-end plus the partial output
projection for its head group; the host sums the two head-group partials
per batch and adds the (bo + Wo@bv) bias once. No collectives.

v5: Q/K/V projections run as fp8e4 DoubleRow matmuls with K=256 per pass
and scaled residual compensation: 32*y = x8@w8 + x8@dw8 + dx8@w8 with
w8 = f8(32*W), dw8 = f8(32*W - w8), dx8 = f8(x - f8(x)). Three passes cost
25% less PE time than bf16's eight K=128 passes while matching bf16
accuracy (~0.14% projection error). The softmax denominator is folded into
the attn@V matmul via a ones-column appended to V (po psum row 64). The
K-projection bias is dropped: a k-independent additive score offset
cancels exactly in softmax. The output projection stays bf16 (attention
outputs are activations; e4m3 storage would cost ~1.5e-2 accuracy). The
exp (33.5M elements/core) is the roofline: split across Activation (exact
Exp) and DVE (Schraudolph fast-exp), the only engines that can read PSUM;
the PE (~60% busy) absorbs pipeline stalls.
"""

from contextlib import ExitStack

import numpy as np
import ml_dtypes

import concourse.bass as bass
import concourse.tile as tile
from concourse import mybir
from concourse.bass_utils import run_bass_kernel_spmd
from concourse.vector_clock import ScopedClock
from bass_rust import InstNoOp, SyncInfo

BF16 = mybir.dt.bfloat16
F32 = mybir.dt.float32
FP8 = mybir.dt.float8e4
AF = mybir.ActivationFunctionType
DR = mybir.MatmulPerfMode.DoubleRow

B, S, D = 4, 2048, 1024
H, HD = 16, 64
GH = 8          # heads per core
GM = GH * HD    # 512 head dims per core
NQB = 4         # q blocks of 512
QB = 512
NKC = 16        # k chunks of 128
NJ = NKC // 2   # k chunk pairs
NC2 = 4         # 256-deep contraction chunks for D=1024
NMT = GM // 128  # 4 m-tiles
VW = 128        # padded stationary width per (j, head) in v8 (64 V + 1 ones)
WS = 32.0       # fp8 weight/residual scale (carried by Q/K/V, undone on host)
ESC = 0.125 / (WS * WS)  # exp scale: scores carry WS^2

_META_TYPES = ("TileBranchInst", "BassTileLoopBlock", "BassTilePoolBoundary")


class _TileCtx(tile.TileContext):
    """Splits multi-sem-wait instructions: the pinned walrus rejects any TPB
    instruction carrying more than one sem-wait, while Tile emits joins and a
    global end-of-context drain with several."""

    def _split_waits(self, ordered):
        nc = self.nc
        for bb_name, insts in ordered.items():
            out = []
            for inst in insts:
                si = inst.sync_info
                if (
                    si is not None
                    and si.on_wait
                    and len(si.on_wait) > 1
                    and type(inst).__name__ not in _META_TYPES
                    and inst.engine != mybir.EngineType.Unassigned
                ):
                    waits = list(si.on_wait)
                    for w in waits[:-1]:
                        nop = InstNoOp(
                            name=nc.get_next_instruction_name(), ins=[], outs=[]
                        )
                        nop.engine = inst.engine
                        nop.sync_info = SyncInfo(on_wait=[w], on_update=[])
                        out.append(nop)
                    inst.sync_info = SyncInfo(
                        on_wait=[waits[-1]], on_update=list(si.on_update)
                    )
                out.append(inst)
            ordered[bb_name] = out

    def _lower_ordered_insts(self, ordered):
        self._split_waits(ordered)
        super()._lower_ordered_insts(ordered)

    def _drain_and_barrier(self, tick_clock, wait_clock):
        drain_inst = self.nc.sync.drain()
        wait_clock.add_sem_waits(
            drain_inst.ins, ScopedClock({None: tick_clock.global_clock})
        )
        si = drain_inst.ins.sync_info
        waits = list(si.on_wait) if si is not None else []
        if len(waits) > 1:
            drain_inst.ins.sync_info = SyncInfo(
                on_wait=waits[:1], on_update=list(si.on_update)
            )
            for w in waits[1:]:
                extra = self.nc.sync.drain()
                extra.ins.sync_info = SyncInfo(on_wait=[w], on_update=[])

        self.nc.all_engine_barrier()
        assert self.sems is not None
        popped = self.nc._tile_sem_poison_stack.pop()
        assert popped is self._sem_poison
        self.nc.clear_and_free_semaphores(list(self.sems.allocated().values()))
        self.nc.all_engine_barrier()


def _build_program():
    nc = bass.Bass(trn_type="TRN2", debug=False, num_devices=8)

    # fp8 operands in the DoubleRow pair layout: n = 256c + 128i + p.
    x8 = nc.dram_tensor("x8", [128, NC2 * 2 * S], FP8, kind="ExternalInput").ap()
    dx8 = nc.dram_tensor("dx8", [128, NC2 * 2 * S], FP8, kind="ExternalInput").ap()
    wq8 = nc.dram_tensor("wq8", [128, NC2 * 2 * GM], FP8, kind="ExternalInput").ap()
    dwq8 = nc.dram_tensor("dwq8", [128, NC2 * 2 * GM], FP8, kind="ExternalInput").ap()
    wk8 = nc.dram_tensor("wk8", [128, NC2 * 2 * GM], FP8, kind="ExternalInput").ap()
    dwk8 = nc.dram_tensor("dwk8", [128, NC2 * 2 * GM], FP8, kind="ExternalInput").ap()
    wv8 = nc.dram_tensor("wv8", [128, NC2 * 2 * GM], FP8, kind="ExternalInput").ap()
    dwv8 = nc.dram_tensor("dwv8", [128, NC2 * 2 * GM], FP8, kind="ExternalInput").ap()
    # bf16 out-projection weights, pair-major: [128, 4 mt x 1024]
    wo = nc.dram_tensor("wo", [128, NMT * D], BF16, kind="ExternalInput").ap()
    bq = nc.dram_tensor("bq", [GM], F32, kind="ExternalInput").ap()
    outT = nc.dram_tensor("outT", [D, S], F32, kind="ExternalOutput").ap()

    with _TileCtx(nc) as tc, ExitStack() as ctx:
        const_pool = ctx.enter_context(tc.tile_pool(name="const", bufs=1))
        act_pool = ctx.enter_context(tc.tile_pool(name="acts", bufs=1))
        qk8_pool = ctx.enter_context(tc.tile_pool(name="qk8", bufs=2))
        slab_pool = ctx.enter_context(tc.tile_pool(name="slab", bufs=20))
        y_pool = ctx.enter_context(tc.tile_pool(name="y", bufs=8))
        s_ps = ctx.enter_context(tc.tile_pool(name="s_ps", bufs=3, space="PSUM"))
        o_ps = ctx.enter_context(tc.tile_pool(name="o_ps", bufs=1, space="PSUM"))
        sm_ps = ctx.enter_context(tc.tile_pool(name="sm_ps", bufs=1, space="PSUM"))

        # ---- weights / inputs / constants --------------------------------
        bq_sb = const_pool.tile([128, NMT], F32, tag="bq")
        nc.sync.dma_start(bq_sb[:], bq.rearrange("(c p) -> p c", p=128))

        wsz = NC2 * 2 * GM
        x_sb = const_pool.tile([128, NC2 * 2 * S], FP8, tag="x8")
        dx_sb = const_pool.tile([128, NC2 * 2 * S], FP8, tag="dx8")
        xv = x_sb[:].rearrange("p (c i s) -> p c i s", i=2, s=S)
        dxv = dx_sb[:].rearrange("p (c i s) -> p c i s", i=2, s=S)
        x8v = x8.rearrange("p (c i s) -> p c i s", i=2, s=S)
        dx8v = dx8.rearrange("p (c i s) -> p c i s", i=2, s=S)
        wk_sb = const_pool.tile([128, wsz], FP8, tag="wk")
        dwk_sb = const_pool.tile([128, wsz], FP8, tag="dwk")
        wq_sb = const_pool.tile([128, wsz], FP8, tag="wq")
        dwq_sb = const_pool.tile([128, wsz], FP8, tag="dwq")
        wkvr = wk_sb[:].rearrange("p (c i m) -> p c i m", i=2, m=GM)
        dwkvr = dwk_sb[:].rearrange("p (c i m) -> p c i m", i=2, m=GM)
        wqvr = wq_sb[:].rearrange("p (c i m) -> p c i m", i=2, m=GM)
        dwqvr = dwq_sb[:].rearrange("p (c i m) -> p c i m", i=2, m=GM)
        wk8v = wk8.rearrange("p (c i m) -> p c i m", i=2, m=GM)
        dwk8v = dwk8.rearrange("p (c i m) -> p c i m", i=2, m=GM)
        wq8v = wq8.rearrange("p (c i m) -> p c i m", i=2, m=GM)
        dwq8v = dwq8.rearrange("p (c i m) -> p c i m", i=2, m=GM)

        # startup-critical loads first: x/dx s-block 0 and the mi=0 weight
        # slices, so the first K/Q projection chains can begin immediately.
        nc.sync.dma_start(xv[:, :, :, 0:QB], x8v[:, :, :, 0:QB])
        nc.sync.dma_start(dxv[:, :, :, 0:QB], dx8v[:, :, :, 0:QB])
        for dst, src in (
            (wkvr, wk8v), (dwkvr, dwk8v), (wqvr, wq8v), (dwqvr, dwq8v),
        ):
            nc.sync.dma_start(dst[:, :, :, 0:128], src[:, :, :, 0:128])
        wv_sb = const_pool.tile([128, wsz], FP8, tag="wv")
        dwv_sb = const_pool.tile([128, wsz], FP8, tag="dwv")

        def load_bulk():
            nc.sync.dma_start(wv_sb[:], wv8[:, :])
            nc.sync.dma_start(dwv_sb[:], dwv8[:, :])
            nc.sync.dma_start(
                xv[:, :, :, QB : 2 * QB], x8v[:, :, :, QB : 2 * QB]
            )
            nc.sync.dma_start(
                dxv[:, :, :, QB : 2 * QB], dx8v[:, :, :, QB : 2 * QB]
            )

        def load_bulk2():
            for sc in range(2, 4):
                nc.sync.dma_start(
                    xv[:, :, :, sc * QB : (sc + 1) * QB],
                    x8v[:, :, :, sc * QB : (sc + 1) * QB],
                )
                nc.sync.dma_start(
                    dxv[:, :, :, sc * QB : (sc + 1) * QB],
                    dx8v[:, :, :, sc * QB : (sc + 1) * QB],
                )

        def load_wrest():
            for dst, src2 in (
                (wkvr, wk8v), (dwkvr, dwk8v), (wqvr, wq8v), (dwqvr, dwq8v),
            ):
                nc.sync.dma_start(dst[:, :, :, 128:GM], src2[:, :, :, 128:GM])
        wo_sb = const_pool.tile([128, NMT * D], BF16, tag="wo")

        def load_wo():
            nc.sync.dma_start(wo_sb[:], wo[:, :])

        wqv = wq_sb[:].rearrange("p (c i m) -> p c i m", i=2, m=GM)
        dwqv = dwq_sb[:].rearrange("p (c i m) -> p c i m", i=2, m=GM)
        wkv = wk_sb[:].rearrange("p (c i m) -> p c i m", i=2, m=GM)
        dwkv = dwk_sb[:].rearrange("p (c i m) -> p c i m", i=2, m=GM)
        wvv = wv_sb[:].rearrange("p (c i m) -> p c i m", i=2, m=GM)
        dwvv = dwv_sb[:].rearrange("p (c i m) -> p c i m", i=2, m=GM)

        ones64 = const_pool.tile([1, 64], BF16, tag="ones64")
        nc.vector.memset(ones64[:], 1.0)
        rd = [
            act_pool.tile([1, QB], BF16, name=f"rd{i}", tag=f"rd{i}")
            for i in range(2)
        ]
        tmp_o = [
            act_pool.tile([64, QB], BF16, name=f"tmpo{i}", tag=f"tmpo{i}")
            for i in range(2)
        ]
        bc_sb = [
            act_pool.tile([64, QB], BF16, name=f"bcs{i}", tag=f"bcs{i}")
            for i in range(2)
        ]

        # ---- persistent activations -------------------------------------
        qtr = act_pool.tile([32, GH * 2 * S], FP8, tag="qtr")
        ktr = act_pool.tile([32, GH * 2 * S], FP8, tag="ktr")
        # v8: [128 keys, 8 jpairs x 8 heads x (2 chunk x 128 pad)] fp8;
        # cols 0:64 = V dims, col 64 = ones (softmax denominator), rest pad.
        v8 = act_pool.tile([128, NJ * GH * 2 * VW], FP8, tag="v8")
        v8v = v8[:].rearrange("p (j h i m) -> p j h i m", h=GH, i=2, m=VW)
        nc.vector.memset(v8v[:, :, :, :, 64:65], 1.0)
        # O^T per head-pair: [128, S] bf16; even head rows 0-63, odd 64-127.
        otp = [
            act_pool.tile([128, S], BF16, name=f"otp{t}", tag=f"otp{t}")
            for t in range(NMT)
        ]

        # Warm the PE p-state so the first projection chains run at full
        # clock: a stream of tiny matmuls on constant data.
        wps = sm_ps.tile([64, 64], F32, name="wps", tag="sm")
        for i in range(56):
            nc.tensor.matmul(
                wps[:], ones64[:], ones64[:], start=True, stop=True,
            )

        # ---- background task closures ------------------------------------
        def kq_chain(wv_, dwv_, mi, c):
            """12 fp8-DR passes: x8@w8 + x8@dw8 + dx8@w8, K=256 each."""
            ps = sm_ps.tile([128, QB], F32, name="kqp", tag="sm")
            msl = slice(mi * 128, (mi + 1) * 128)
            csl = slice(c * QB, (c + 1) * QB)
            n = 0
            for dc in range(NC2):
                for lhs, rhs in (
                    (wv_[:, dc, :, msl], xv[:, dc, :, csl]),
                    (dwv_[:, dc, :, msl], xv[:, dc, :, csl]),
                    (wv_[:, dc, :, msl], dxv[:, dc, :, csl]),
                ):
                    n += 1
                    nc.tensor.matmul(
                        ps[:], lhs, rhs,
                        start=(n == 1),
                        stop=(n == 3 * NC2),
                        perf_mode=DR,
                    )
            return ps

        def kq_dma(dst, q8, mi, c0, w):
            """Rearrange q8 [128, w] into the pair-split [32, h, i, s] layout
            of qtr/ktr; 4 plain-partition-range DMAs."""
            dv = dst[:].rearrange("p (h i s) -> p h i s", i=2, s=S)
            for hh in range(2):
                for i in range(2):
                    h = 2 * mi + hh
                    nc.sync.dma_start(
                        dv[:, h, i, c0 * QB : c0 * QB + w],
                        q8[hh * 64 + i * 32 : hh * 64 + i * 32 + 32, 0:w],
                    )

        def kq_col(wv_, dwv_, dst, mi, c, bias):
            """mi=0 path: single 512-col block + immediate rearrange DMAs."""
            ps = kq_chain(wv_, dwv_, mi, c)
            q8 = qk8_pool.tile([128, QB], FP8, name="q8blk", tag="q8")
            with tc.high_priority(offset=150):
                if bias:
                    nc.vector.tensor_scalar_add(
                        q8[:], ps[:], bq_sb[:, mi : mi + 1]
                    )
                else:
                    nc.vector.tensor_copy(q8[:], ps[:])
            kq_dma(dst, q8, mi, c, QB)

        kq_full = {}

        def kq_col_wide(wv_, dwv_, dst, mi, c, bias):
            """mi>=1 path: accumulate 4 col-blocks into one [128, 2048] tile,
            then 4 merged DMAs when the last block lands."""
            key = (dst is qtr, mi)
            if c == 0:
                kq_full[key] = qk8_pool.tile(
                    [128, S], FP8, name="q8full", tag="q8f"
                )
            q8 = kq_full[key]
            ps = kq_chain(wv_, dwv_, mi, c)
            with tc.high_priority(offset=150):
                if bias:
                    nc.vector.tensor_scalar_add(
                        q8[:, c * QB : (c + 1) * QB], ps[:],
                        bq_sb[:, mi : mi + 1],
                    )
                else:
                    nc.vector.tensor_copy(q8[:, c * QB : (c + 1) * QB], ps[:])
            if c == NC2 - 1:
                kq_dma(dst, q8, mi, 0, S)

        def v_proj(si):
            """V projection for keys chunk si, all 8 heads at once."""
            j, par = si // 2, si % 2
            pool = (sm_ps, s_ps, o_ps)[si % 3]
            ps = pool.tile([128, GM], F32, name="vp",
                           tag=("sm", "s2", "po")[si % 3])
            csl = slice(si * 128, (si + 1) * 128)
            n = 0
            for dc in range(NC2):
                for lhs, rhs in (
                    (xv[:, dc, :, csl], wvv[:, dc, :, :]),
                    (xv[:, dc, :, csl], dwvv[:, dc, :, :]),
                    (dxv[:, dc, :, csl], wvv[:, dc, :, :]),
                ):
                    n += 1
                    nc.tensor.matmul(
                        ps[:], lhs, rhs,
                        start=(n == 1),
                        stop=(n == 3 * NC2),
                        perf_mode=DR,
                    )
            with tc.high_priority(offset=150):
                nc.vector.tensor_copy(
                    v8v[:, j : j + 1, :, par : par + 1, 0:64],
                    ps[:].rearrange("p (h m) -> p h m", h=GH),
                )

        def out_proj(qb, ec, pool=None):
            pool = pool or sm_ps
            ps = pool.tile([128, QB], F32, name="yp",
                           tag="sm" if pool is sm_ps else ("po" if pool is o_ps else "s2"))
            for mt in range(NMT):
                nc.tensor.matmul(
                    ps[:],
                    wo_sb[:, mt * D + ec * 128 : mt * D + (ec + 1) * 128],
                    otp[mt][:, qb * QB : (qb + 1) * QB],
                    start=(mt == 0),
                    stop=(mt == NMT - 1),
                )
            y_sb = y_pool.tile([128, QB], F32, name="yblk", tag="y")
            with tc.high_priority(offset=150):
                nc.vector.tensor_copy(y_sb[:], ps[:])
            nc.sync.dma_start(
                outT[ec * 128 : (ec + 1) * 128, qb * QB : (qb + 1) * QB], y_sb[:]
            )

        # ---- attention unit emitters -------------------------------------
        # DVE fast-exp: e4m3 bits of exp(s*ESC) are an affine function of s
        # (linear-mantissa log2 trick): bits = s*ESC*8/ln2 + 56 + c.
        FEXP_A = ESC * 8.0 / float(np.log(2.0))
        FEXP_B = 55.55  # HW convert rounds-to-nearest; Schraudolph-centered

        def att_scores(t, qb, hh, j, on_dve):
            """Scores + exp for head 2t+hh at (qb, kc-pair j)."""
            h = 2 * t + hh
            with tc.high_priority(offset=300):
                s2 = s_ps.tile([128, 2 * QB], F32, name="s2", tag="s2")
                qv = qtr[:, h * 2 * S : (h + 1) * 2 * S].rearrange(
                    "p (i s) -> p i s", i=2
                )[:, :, qb * QB : (qb + 1) * QB]
                for par in range(2):
                    kc = 2 * j + par
                    kv = ktr[:, h * 2 * S : (h + 1) * 2 * S].rearrange(
                        "p (i s) -> p i s", i=2
                    )[:, :, kc * 128 : (kc + 1) * 128]
                    nc.tensor.matmul(
                        s2[:, par * QB : (par + 1) * QB],
                        kv, qv, start=True, stop=True, perf_mode=DR,
                    )
                sl = slab_pool.tile([128, 2 * QB], FP8, name="sl", tag="slab")
                if on_dve:
                    nc.vector.tensor_scalar(
                        sl[:].bitcast(mybir.dt.uint8),
                        s2[:],
                        FEXP_A,
                        FEXP_B,
                        mybir.AluOpType.mult,
                        mybir.AluOpType.add,
                    )
                else:
                    nc.scalar.activation(sl[:], s2[:], AF.Exp, scale=ESC)
            return sl

        state = {"po": None, "n": 0}

        def att_consume(t, qb, hh, j, sl):
            """attn@V (+ folded denominator) for unit (t, qb, hh, j).
            Returns a deferred normalization closure at group end so the PE's
            bc matmul is emitted well after the reciprocal it waits on."""
            h = 2 * t + hh
            if j == 0:
                state["po"] = o_ps.tile([65, QB], F32, name="po", tag="po")
            po = state["po"]
            rhs = sl[:].rearrange("p (i q) -> p i q", i=2)
            lhsT = v8v[:, j, h, :, 0:65]
            nc.tensor.matmul(
                po[:, :], lhsT, rhs,
                start=(j == 0), stop=(j == NJ - 1), perf_mode=DR,
            )
            if j != NJ - 1:
                return None
            k = state["n"] % 2
            state["n"] += 1
            r = rd[k]
            with tc.high_priority(offset=150):
                with nc.allow_low_precision(reason="softmax recip in bf16"):
                    nc.vector.reciprocal(r[0:1, :], po[64:65, :])

            def norm():
                bc = s_ps.tile([64, QB], F32, name="bc", tag="s2")
                nc.tensor.matmul(bc[:], ones64[:], r[:], start=True, stop=True)
                bcs = bc_sb[k]
                nc.vector.tensor_copy(bcs[:], bc[:])
                if hh == 0:
                    nc.vector.tensor_mul(
                        otp[t][0:64, qb * QB : (qb + 1) * QB], po[0:64, :], bcs[:]
                    )
                else:
                    tm = tmp_o[k]
                    nc.vector.tensor_mul(tm[:], po[0:64, :], bcs[:])
                    nc.sync.dma_start(
                        otp[t][64:128, qb * QB : (qb + 1) * QB], tm[:]
                    )

            return norm

        # ---- the flat schedule -------------------------------------------
        kq_col(wkv, dwkv, ktr, 0, 0, bias=False)
        kq_col(wqv, dwqv, qtr, 0, 0, bias=True)
        load_bulk()

        bg = {}

        def bg_add(u, fn, *args):
            bg.setdefault(u, []).append((fn, args))

        # V for keys chunk si: needed by consume of unit j=si//2 (iter +LAG).
        bg_add(0, v_proj, 0)
        bg_add(0, v_proj, 1)
        for k in range(1, NJ):
            bg_add(k, v_proj, 2 * k)
            bg_add(k, v_proj, 2 * k + 1)
        # remaining K0/Q0 column blocks: scores at iteration j read K cols
        # kc=2j,2j+1 (c-block j//2); Q c-block c is read from iteration 16c.
        for c in (1, 2, 3):
            bg_add(2 * c - 1, kq_col, wkv, dwkv, ktr, 0, c, False)
            bg_add(2 * c, kq_col, wqv, dwqv, qtr, 0, c, True)
        bg_add(1, load_bulk2)
        bg_add(8, load_wrest)
        bg_add(24, load_wo)
        # K/Q projections for m-tile mi: spread over window mi-1, merged DMA.
        for mi in range(1, NMT):
            base = 64 * (mi - 1)
            for c in range(4):
                bg_add(base + 24 + 8 * c, kq_col_wide, wkv, dwkv, ktr, mi, c, False)
                bg_add(base + 28 + 8 * c, kq_col_wide, wqv, dwqv, qtr, mi, c, True)
        # output projection blocks, spread after each qb's last otp write
        for qb in range(NQB - 1):
            for ec in range(8):
                bg_add(214 + 16 * qb + 2 * ec, out_proj, qb, ec)

        units = [
            (t, qb, hh, j)
            for t in range(NMT)
            for qb in range(NQB)
            for hh in range(2)
            for j in range(NJ)
        ]
        # consume lag: unit u's attn@V runs after scores of unit u+LAG, so
        # the exp engines get LAG units of PE work to hide their latency.
        LAG = 2
        # Engine-balanced exp split: 72 units on DVE fast-exp (Bresenham-
        # spread so Act/DVE exps overlap instead of serializing), the rest
        # exact Exp on Act. The first window keeps DVE free for the
        # V-projection copies.
        DVE_EXP = set(
            u for u in range(len(units))
            if u >= 16 and (u * 9) // 32 != ((u - 1) * 9) // 32
        )
        pending = []
        deferred = {}
        NU = len(units)
        for u in range(NU + LAG + 2):
            if u < NU:
                t, qb, hh, j = units[u]
                sl = att_scores(t, qb, hh, j, u in DVE_EXP)
                pending.append((t, qb, hh, j, sl))
            for fn, args in bg.pop(u, []):
                fn(*args)
            nrm = deferred.pop(u, None)
            if nrm is not None:
                nrm()
            while pending:
                thr = 0
                if u < NU:
                    # hold each group's first consume two extra iterations so
                    # the single po bank has been read by the previous
                    # group's normalization multiply; hold group 0 much
                    # longer so o_ps can host early V-projection chains
                    if pending[0][:4] == (0, 0, 0, 0):
                        thr = 16
                    else:
                        thr = LAG + (2 if pending[0][3] == 0 else 0)
                if len(pending) <= thr:
                    break
                nrm = att_consume(*pending.pop(0))
                if nrm is not None:
                    if u < NU:
                        deferred[u + 1] = nrm
                    else:
                        # drain: emit immediately so the next group's j0
                        # never precedes this group's normalization in the
                        # PE stream (in-order cycle hazard)
                        nrm()
                if u < NU:
                    break
        # drain: leftover norms, bg tasks scheduled past the end, then the
        # last qb's output projection
        for u2 in sorted(deferred.keys()):
            deferred[u2]()
        deferred.clear()
        for u in sorted(bg.keys()):
            for fn, args in bg[u]:
                fn(*args)
        for ec in range(8):
            out_proj(NQB - 1, ec, pool=(sm_ps, s_ps, o_ps)[ec % 3])

    return nc


_NC = None


def _get_program():
    global _NC
    if _NC is None:
        _NC = _build_program()
    return _NC


_F8 = ml_dtypes.float8_e4m3


def _pair(a, chunks):
    """[N, M] -> DoubleRow pair layout [128, chunks*2*M], n = 256c+128i+p."""
    n, m = a.shape
    assert n == chunks * 256
    return np.ascontiguousarray(
        a.reshape(chunks, 2, 128, m).transpose(2, 0, 1, 3).reshape(128, -1)
    )


def _q8_with_resid(a):
    """fp8 value + fp8 residual of a contiguous array."""
    a8 = a.astype(_F8)
    da8 = (a - a8.astype(np.float32)).astype(_F8)
    return a8, da8


def kernel(x, Wq, bq, Wk, bk, Wv, bv, Wo, bo):
    bf = ml_dtypes.bfloat16
    x = np.asarray(x, np.float32)
    Wq = np.asarray(Wq, np.float32)
    Wk = np.asarray(Wk, np.float32)
    Wv = np.asarray(Wv, np.float32)
    Wo = np.asarray(Wo, np.float32)
    bq = np.asarray(bq, np.float32)
    bv = np.asarray(bv, np.float32)
    bo = np.asarray(bo, np.float32)

    in_maps = []
    for c in range(8):
        b, g = c // 2, c % 2
        sl = slice(g * GM, (g + 1) * GM)
        x8, dx8 = _q8_with_resid(_pair(x[b].T, NC2))
        wq8, dwq8 = _q8_with_resid(_pair(Wq[sl, :].T * WS, NC2))
        wk8, dwk8 = _q8_with_resid(_pair(Wk[sl, :].T * WS, NC2))
        wv8, dwv8 = _q8_with_resid(_pair(Wv[sl, :].T * WS, NC2))
        wo_slice = Wo[:, sl].T  # [512, 1024]
        in_maps.append(
            {
                "x8": x8, "dx8": dx8,
                "wq8": wq8, "dwq8": dwq8,
                "wk8": wk8, "dwk8": dwk8,
                "wv8": wv8, "dwv8": dwv8,
                "wo": np.ascontiguousarray(
                    wo_slice.reshape(NMT, 128, D).transpose(1, 0, 2).reshape(
                        128, NMT * D
                    )
                ).astype(bf),
                "bq": np.ascontiguousarray(bq[sl] * WS),
            }
        )

    nc = _get_program()
    res = run_bass_kernel_spmd(nc, in_maps, core_ids=list(range(8)))
    # host-side bias: out = attn_hat @ Wo.T + (bo + Wo @ bv); the V bias bv
    # passes through softmax-weighted averaging unchanged (weights sum to 1).
    # The on-core result carries the WS scale from V.
    bo_full = bo + Wo @ bv
    out = np.empty((B, S, D), np.float32)
    for b in range(B):
        acc = res.results[2 * b]["outT"].astype(np.float32) + res.results[
            2 * b + 1
        ]["outT"].astype(np.float32)
        out[b] = acc.T / WS + bo_full[None, :]
    return out


# revision 27
# speedup vs baseline: 1.0046x; 1.0046x over previous
"""Multi-head self-attention (B=4, S=2048, D=1024, H=16, Hd=64) on 8 TRN2 cores.

Sharding: core c -> (batch b = c//2, head-group g = c%2 of 8 heads).
Each core computes its batch's 8 heads end-to-end plus the partial output
projection for its head group; the host sums the two head-group partials
per batch and adds the (bo + Wo@bv) bias once. No collectives.

v5: Q/K/V projections run as fp8e4 DoubleRow matmuls with K=256 per pass
and scaled residual compensation: 32*y = x8@w8 + x8@dw8 + dx8@w8 with
w8 = f8(32*W), dw8 = f8(32*W - w8), dx8 = f8(x - f8(x)). Three passes cost
25% less PE time than bf16's eight K=128 passes while matching bf16
accuracy (~0.14% projection error). The softmax denominator is folded into
the attn@V matmul via a ones-column appended to V (po psum row 64). The
K-projection bias is dropped: a k-independent additive score offset
cancels exactly in softmax. The output projection stays bf16 (attention
outputs are activations; e4m3 storage would cost ~1.5e-2 accuracy). The
exp (33.5M elements/core) is the roofline: split across Activation (exact
Exp) and DVE (Schraudolph fast-exp), the only engines that can read PSUM;
the PE (~60% busy) absorbs pipeline stalls.
"""

from contextlib import ExitStack

import numpy as np
import ml_dtypes

import concourse.bass as bass
import concourse.tile as tile
from concourse import mybir
from concourse.bass_utils import run_bass_kernel_spmd
from concourse.vector_clock import ScopedClock
from bass_rust import InstNoOp, SyncInfo

BF16 = mybir.dt.bfloat16
F32 = mybir.dt.float32
FP8 = mybir.dt.float8e4
AF = mybir.ActivationFunctionType
DR = mybir.MatmulPerfMode.DoubleRow

B, S, D = 4, 2048, 1024
H, HD = 16, 64
GH = 8          # heads per core
GM = GH * HD    # 512 head dims per core
NQB = 4         # q blocks of 512
QB = 512
NKC = 16        # k chunks of 128
NJ = NKC // 2   # k chunk pairs
NC2 = 4         # 256-deep contraction chunks for D=1024
NMT = GM // 128  # 4 m-tiles
VW = 128        # padded stationary width per (j, head) in v8 (64 V + 1 ones)
WS = 32.0       # fp8 weight/residual scale (carried by Q/K/V, undone on host)
ESC = 0.125 / (WS * WS)  # exp scale: scores carry WS^2

_META_TYPES = ("TileBranchInst", "BassTileLoopBlock", "BassTilePoolBoundary")


class _TileCtx(tile.TileContext):
    """Splits multi-sem-wait instructions: the pinned walrus rejects any TPB
    instruction carrying more than one sem-wait, while Tile emits joins and a
    global end-of-context drain with several."""

    def _split_waits(self, ordered):
        nc = self.nc
        for bb_name, insts in ordered.items():
            out = []
            for inst in insts:
                si = inst.sync_info
                if (
                    si is not None
                    and si.on_wait
                    and len(si.on_wait) > 1
                    and type(inst).__name__ not in _META_TYPES
                    and inst.engine != mybir.EngineType.Unassigned
                ):
                    waits = list(si.on_wait)
                    for w in waits[:-1]:
                        nop = InstNoOp(
                            name=nc.get_next_instruction_name(), ins=[], outs=[]
                        )
                        nop.engine = inst.engine
                        nop.sync_info = SyncInfo(on_wait=[w], on_update=[])
                        out.append(nop)
                    inst.sync_info = SyncInfo(
                        on_wait=[waits[-1]], on_update=list(si.on_update)
                    )
                out.append(inst)
            ordered[bb_name] = out

    def _lower_ordered_insts(self, ordered):
        self._split_waits(ordered)
        super()._lower_ordered_insts(ordered)

    def _drain_and_barrier(self, tick_clock, wait_clock):
        drain_inst = self.nc.sync.drain()
        wait_clock.add_sem_waits(
            drain_inst.ins, ScopedClock({None: tick_clock.global_clock})
        )
        si = drain_inst.ins.sync_info
        waits = list(si.on_wait) if si is not None else []
        if len(waits) > 1:
            drain_inst.ins.sync_info = SyncInfo(
                on_wait=waits[:1], on_update=list(si.on_update)
            )
            for w in waits[1:]:
                extra = self.nc.sync.drain()
                extra.ins.sync_info = SyncInfo(on_wait=[w], on_update=[])

        self.nc.all_engine_barrier()
        assert self.sems is not None
        popped = self.nc._tile_sem_poison_stack.pop()
        assert popped is self._sem_poison
        self.nc.clear_and_free_semaphores(list(self.sems.allocated().values()))
        self.nc.all_engine_barrier()


def _build_program():
    nc = bass.Bass(trn_type="TRN2", debug=False, num_devices=8)

    # fp8 operands in the DoubleRow pair layout: n = 256c + 128i + p.
    x8 = nc.dram_tensor("x8", [128, NC2 * 2 * S], FP8, kind="ExternalInput").ap()
    dx8 = nc.dram_tensor("dx8", [128, NC2 * 2 * S], FP8, kind="ExternalInput").ap()
    wq8 = nc.dram_tensor("wq8", [128, NC2 * 2 * GM], FP8, kind="ExternalInput").ap()
    dwq8 = nc.dram_tensor("dwq8", [128, NC2 * 2 * GM], FP8, kind="ExternalInput").ap()
    wk8 = nc.dram_tensor("wk8", [128, NC2 * 2 * GM], FP8, kind="ExternalInput").ap()
    dwk8 = nc.dram_tensor("dwk8", [128, NC2 * 2 * GM], FP8, kind="ExternalInput").ap()
    wv8 = nc.dram_tensor("wv8", [128, NC2 * 2 * GM], FP8, kind="ExternalInput").ap()
    dwv8 = nc.dram_tensor("dwv8", [128, NC2 * 2 * GM], FP8, kind="ExternalInput").ap()
    # bf16 out-projection weights, pair-major: [128, 4 mt x 1024]
    wo = nc.dram_tensor("wo", [128, NMT * D], BF16, kind="ExternalInput").ap()
    bq = nc.dram_tensor("bq", [GM], F32, kind="ExternalInput").ap()
    outT = nc.dram_tensor("outT", [D, S], F32, kind="ExternalOutput").ap()

    with _TileCtx(nc) as tc, ExitStack() as ctx:
        const_pool = ctx.enter_context(tc.tile_pool(name="const", bufs=1))
        act_pool = ctx.enter_context(tc.tile_pool(name="acts", bufs=1))
        qk8_pool = ctx.enter_context(tc.tile_pool(name="qk8", bufs=2))
        slab_pool = ctx.enter_context(tc.tile_pool(name="slab", bufs=8))
        y_pool = ctx.enter_context(tc.tile_pool(name="y", bufs=8))
        s_ps = ctx.enter_context(tc.tile_pool(name="s_ps", bufs=3, space="PSUM"))
        o_ps = ctx.enter_context(tc.tile_pool(name="o_ps", bufs=1, space="PSUM"))
        sm_ps = ctx.enter_context(tc.tile_pool(name="sm_ps", bufs=1, space="PSUM"))

        # ---- weights / inputs / constants --------------------------------
        bq_sb = const_pool.tile([128, NMT], F32, tag="bq")
        nc.sync.dma_start(bq_sb[:], bq.rearrange("(c p) -> p c", p=128))

        wsz = NC2 * 2 * GM
        x_sb = const_pool.tile([128, NC2 * 2 * S], FP8, tag="x8")
        dx_sb = const_pool.tile([128, NC2 * 2 * S], FP8, tag="dx8")
        xv = x_sb[:].rearrange("p (c i s) -> p c i s", i=2, s=S)
        dxv = dx_sb[:].rearrange("p (c i s) -> p c i s", i=2, s=S)
        x8v = x8.rearrange("p (c i s) -> p c i s", i=2, s=S)
        dx8v = dx8.rearrange("p (c i s) -> p c i s", i=2, s=S)
        wk_sb = const_pool.tile([128, wsz], FP8, tag="wk")
        dwk_sb = const_pool.tile([128, wsz], FP8, tag="dwk")
        wq_sb = const_pool.tile([128, wsz], FP8, tag="wq")
        dwq_sb = const_pool.tile([128, wsz], FP8, tag="dwq")
        wkvr = wk_sb[:].rearrange("p (c i m) -> p c i m", i=2, m=GM)
        dwkvr = dwk_sb[:].rearrange("p (c i m) -> p c i m", i=2, m=GM)
        wqvr = wq_sb[:].rearrange("p (c i m) -> p c i m", i=2, m=GM)
        dwqvr = dwq_sb[:].rearrange("p (c i m) -> p c i m", i=2, m=GM)
        wk8v = wk8.rearrange("p (c i m) -> p c i m", i=2, m=GM)
        dwk8v = dwk8.rearrange("p (c i m) -> p c i m", i=2, m=GM)
        wq8v = wq8.rearrange("p (c i m) -> p c i m", i=2, m=GM)
        dwq8v = dwq8.rearrange("p (c i m) -> p c i m", i=2, m=GM)

        # startup-critical loads first: x/dx s-block 0 and the mi=0 weight
        # slices, so the first K/Q projection chains can begin immediately.
        nc.sync.dma_start(xv[:, :, :, 0:QB], x8v[:, :, :, 0:QB])
        nc.sync.dma_start(dxv[:, :, :, 0:QB], dx8v[:, :, :, 0:QB])
        for dst, src in (
            (wkvr, wk8v), (dwkvr, dwk8v), (wqvr, wq8v), (dwqvr, dwq8v),
        ):
            nc.sync.dma_start(dst[:, :, :, 0:128], src[:, :, :, 0:128])
        wv_sb = const_pool.tile([128, wsz], FP8, tag="wv")
        dwv_sb = const_pool.tile([128, wsz], FP8, tag="dwv")

        def load_bulk():
            nc.sync.dma_start(wv_sb[:], wv8[:, :])
            nc.sync.dma_start(dwv_sb[:], dwv8[:, :])
            nc.sync.dma_start(
                xv[:, :, :, QB : 2 * QB], x8v[:, :, :, QB : 2 * QB]
            )
            nc.sync.dma_start(
                dxv[:, :, :, QB : 2 * QB], dx8v[:, :, :, QB : 2 * QB]
            )

        def load_bulk2():
            for sc in range(2, 4):
                nc.sync.dma_start(
                    xv[:, :, :, sc * QB : (sc + 1) * QB],
                    x8v[:, :, :, sc * QB : (sc + 1) * QB],
                )
                nc.sync.dma_start(
                    dxv[:, :, :, sc * QB : (sc + 1) * QB],
                    dx8v[:, :, :, sc * QB : (sc + 1) * QB],
                )

        def load_wrest():
            for dst, src2 in (
                (wkvr, wk8v), (dwkvr, dwk8v), (wqvr, wq8v), (dwqvr, dwq8v),
            ):
                nc.sync.dma_start(dst[:, :, :, 128:GM], src2[:, :, :, 128:GM])
        wo_sb = const_pool.tile([128, NMT * D], BF16, tag="wo")

        def load_wo():
            nc.sync.dma_start(wo_sb[:], wo[:, :])

        wqv = wq_sb[:].rearrange("p (c i m) -> p c i m", i=2, m=GM)
        dwqv = dwq_sb[:].rearrange("p (c i m) -> p c i m", i=2, m=GM)
        wkv = wk_sb[:].rearrange("p (c i m) -> p c i m", i=2, m=GM)
        dwkv = dwk_sb[:].rearrange("p (c i m) -> p c i m", i=2, m=GM)
        wvv = wv_sb[:].rearrange("p (c i m) -> p c i m", i=2, m=GM)
        dwvv = dwv_sb[:].rearrange("p (c i m) -> p c i m", i=2, m=GM)

        ones64 = const_pool.tile([1, 64], BF16, tag="ones64")
        nc.vector.memset(ones64[:], 1.0)
        rd = [
            act_pool.tile([1, QB], BF16, name=f"rd{i}", tag=f"rd{i}")
            for i in range(2)
        ]
        tmp_o = [
            act_pool.tile([64, QB], BF16, name=f"tmpo{i}", tag=f"tmpo{i}")
            for i in range(2)
        ]
        bc_sb = [
            act_pool.tile([64, QB], BF16, name=f"bcs{i}", tag=f"bcs{i}")
            for i in range(2)
        ]

        # ---- persistent activations -------------------------------------
        qtr = act_pool.tile([32, GH * 2 * S], FP8, tag="qtr")
        ktr = act_pool.tile([32, GH * 2 * S], FP8, tag="ktr")
        # v8: [128 keys, 8 jpairs x 8 heads x (2 chunk x 128 pad)] fp8;
        # cols 0:64 = V dims, col 64 = ones (softmax denominator), rest pad.
        v8 = act_pool.tile([128, NJ * GH * 2 * VW], FP8, tag="v8")
        v8v = v8[:].rearrange("p (j h i m) -> p j h i m", h=GH, i=2, m=VW)
        nc.vector.memset(v8v[:, :, :, :, 64:65], 1.0)
        # O^T per head-pair: [128, S] bf16; even head rows 0-63, odd 64-127.
        otp = [
            act_pool.tile([128, S], BF16, name=f"otp{t}", tag=f"otp{t}")
            for t in range(NMT)
        ]

        # Warm the PE p-state so the first projection chains run at full
        # clock: a stream of tiny matmuls on constant data.
        wps = sm_ps.tile([64, 64], F32, name="wps", tag="sm")
        for i in range(56):
            nc.tensor.matmul(
                wps[:], ones64[:], ones64[:], start=True, stop=True,
            )

        # ---- background task closures ------------------------------------
        def kq_chain(wv_, dwv_, mi, c):
            """12 fp8-DR passes: x8@w8 + x8@dw8 + dx8@w8, K=256 each."""
            ps = sm_ps.tile([128, QB], F32, name="kqp", tag="sm")
            msl = slice(mi * 128, (mi + 1) * 128)
            csl = slice(c * QB, (c + 1) * QB)
            n = 0
            for dc in range(NC2):
                for lhs, rhs in (
                    (wv_[:, dc, :, msl], xv[:, dc, :, csl]),
                    (dwv_[:, dc, :, msl], xv[:, dc, :, csl]),
                    (wv_[:, dc, :, msl], dxv[:, dc, :, csl]),
                ):
                    n += 1
                    nc.tensor.matmul(
                        ps[:], lhs, rhs,
                        start=(n == 1),
                        stop=(n == 3 * NC2),
                        perf_mode=DR,
                    )
            return ps

        def kq_dma(dst, q8, mi, c0, w):
            """Rearrange q8 [128, w] into the pair-split [32, h, i, s] layout
            of qtr/ktr; 4 plain-partition-range DMAs."""
            dv = dst[:].rearrange("p (h i s) -> p h i s", i=2, s=S)
            for hh in range(2):
                for i in range(2):
                    h = 2 * mi + hh
                    nc.sync.dma_start(
                        dv[:, h, i, c0 * QB : c0 * QB + w],
                        q8[hh * 64 + i * 32 : hh * 64 + i * 32 + 32, 0:w],
                    )

        def kq_col(wv_, dwv_, dst, mi, c, bias):
            """mi=0 path: single 512-col block + immediate rearrange DMAs."""
            ps = kq_chain(wv_, dwv_, mi, c)
            q8 = qk8_pool.tile([128, QB], FP8, name="q8blk", tag="q8")
            with tc.high_priority(offset=150):
                if bias:
                    nc.vector.tensor_scalar_add(
                        q8[:], ps[:], bq_sb[:, mi : mi + 1]
                    )
                else:
                    nc.vector.tensor_copy(q8[:], ps[:])
            kq_dma(dst, q8, mi, c, QB)

        kq_full = {}

        def kq_col_wide(wv_, dwv_, dst, mi, c, bias):
            """mi>=1 path: accumulate 4 col-blocks into one [128, 2048] tile,
            then 4 merged DMAs when the last block lands."""
            key = (dst is qtr, mi)
            if c == 0:
                kq_full[key] = qk8_pool.tile(
                    [128, S], FP8, name="q8full", tag="q8f"
                )
            q8 = kq_full[key]
            ps = kq_chain(wv_, dwv_, mi, c)
            with tc.high_priority(offset=150):
                if bias:
                    nc.vector.tensor_scalar_add(
                        q8[:, c * QB : (c + 1) * QB], ps[:],
                        bq_sb[:, mi : mi + 1],
                    )
                else:
                    nc.vector.tensor_copy(q8[:, c * QB : (c + 1) * QB], ps[:])
            if c == NC2 - 1:
                kq_dma(dst, q8, mi, 0, S)

        def v_proj(si):
            """V projection for keys chunk si, all 8 heads at once."""
            j, par = si // 2, si % 2
            pool = s_ps if si % 2 else sm_ps
            ps = pool.tile([128, GM], F32, name="vp",
                           tag="s2" if si % 2 else "sm")
            csl = slice(si * 128, (si + 1) * 128)
            n = 0
            for dc in range(NC2):
                for lhs, rhs in (
                    (xv[:, dc, :, csl], wvv[:, dc, :, :]),
                    (xv[:, dc, :, csl], dwvv[:, dc, :, :]),
                    (dxv[:, dc, :, csl], wvv[:, dc, :, :]),
                ):
                    n += 1
                    nc.tensor.matmul(
                        ps[:], lhs, rhs,
                        start=(n == 1),
                        stop=(n == 3 * NC2),
                        perf_mode=DR,
                    )
            with tc.high_priority(offset=150):
                nc.vector.tensor_copy(
                    v8v[:, j : j + 1, :, par : par + 1, 0:64],
                    ps[:].rearrange("p (h m) -> p h m", h=GH),
                )

        def out_proj(qb, ec, pool=None):
            pool = pool or sm_ps
            ps = pool.tile([128, QB], F32, name="yp",
                           tag="sm" if pool is sm_ps else ("po" if pool is o_ps else "s2"))
            for mt in range(NMT):
                nc.tensor.matmul(
                    ps[:],
                    wo_sb[:, mt * D + ec * 128 : mt * D + (ec + 1) * 128],
                    otp[mt][:, qb * QB : (qb + 1) * QB],
                    start=(mt == 0),
                    stop=(mt == NMT - 1),
                )
            y_sb = y_pool.tile([128, QB], F32, name="yblk", tag="y")
            with tc.high_priority(offset=150):
                nc.vector.tensor_copy(y_sb[:], ps[:])
            nc.sync.dma_start(
                outT[ec * 128 : (ec + 1) * 128, qb * QB : (qb + 1) * QB], y_sb[:]
            )

        # ---- attention unit emitters -------------------------------------
        # DVE fast-exp: e4m3 bits of exp(s*ESC) are an affine function of s
        # (linear-mantissa log2 trick): bits = s*ESC*8/ln2 + 56 + c.
        FEXP_A = ESC * 8.0 / float(np.log(2.0))
        FEXP_B = 55.55  # HW convert rounds-to-nearest; Schraudolph-centered

        def att_scores(t, qb, hh, j, on_dve):
            """Scores + exp for head 2t+hh at (qb, kc-pair j)."""
            h = 2 * t + hh
            with tc.high_priority(offset=300):
                s2 = s_ps.tile([128, 2 * QB], F32, name="s2", tag="s2")
                qv = qtr[:, h * 2 * S : (h + 1) * 2 * S].rearrange(
                    "p (i s) -> p i s", i=2
                )[:, :, qb * QB : (qb + 1) * QB]
                for par in range(2):
                    kc = 2 * j + par
                    kv = ktr[:, h * 2 * S : (h + 1) * 2 * S].rearrange(
                        "p (i s) -> p i s", i=2
                    )[:, :, kc * 128 : (kc + 1) * 128]
                    nc.tensor.matmul(
                        s2[:, par * QB : (par + 1) * QB],
                        kv, qv, start=True, stop=True, perf_mode=DR,
                    )
                sl = slab_pool.tile([128, 2 * QB], FP8, name="sl", tag="slab")
                if on_dve:
                    nc.vector.tensor_scalar(
                        sl[:].bitcast(mybir.dt.uint8),
                        s2[:],
                        FEXP_A,
                        FEXP_B,
                        mybir.AluOpType.mult,
                        mybir.AluOpType.add,
                    )
                else:
                    nc.scalar.activation(sl[:], s2[:], AF.Exp, scale=ESC)
            return sl

        state = {"po": None, "n": 0}

        def att_consume(t, qb, hh, j, sl):
            """attn@V (+ folded denominator) for unit (t, qb, hh, j).
            Returns a deferred normalization closure at group end so the PE's
            bc matmul is emitted well after the reciprocal it waits on."""
            h = 2 * t + hh
            if j == 0:
                state["po"] = o_ps.tile([65, QB], F32, name="po", tag="po")
            po = state["po"]
            rhs = sl[:].rearrange("p (i q) -> p i q", i=2)
            lhsT = v8v[:, j, h, :, 0:65]
            nc.tensor.matmul(
                po[:, :], lhsT, rhs,
                start=(j == 0), stop=(j == NJ - 1), perf_mode=DR,
            )
            if j != NJ - 1:
                return None
            k = state["n"] % 2
            state["n"] += 1
            r = rd[k]
            with tc.high_priority(offset=150):
                with nc.allow_low_precision(reason="softmax recip in bf16"):
                    nc.vector.reciprocal(r[0:1, :], po[64:65, :])

            def norm():
                bc = s_ps.tile([64, QB], F32, name="bc", tag="s2")
                nc.tensor.matmul(bc[:], ones64[:], r[:], start=True, stop=True)
                bcs = bc_sb[k]
                nc.vector.tensor_copy(bcs[:], bc[:])
                if hh == 0:
                    nc.vector.tensor_mul(
                        otp[t][0:64, qb * QB : (qb + 1) * QB], po[0:64, :], bcs[:]
                    )
                else:
                    tm = tmp_o[k]
                    nc.vector.tensor_mul(tm[:], po[0:64, :], bcs[:])
                    nc.sync.dma_start(
                        otp[t][64:128, qb * QB : (qb + 1) * QB], tm[:]
                    )

            return norm

        # ---- the flat schedule -------------------------------------------
        kq_col(wkv, dwkv, ktr, 0, 0, bias=False)
        kq_col(wqv, dwqv, qtr, 0, 0, bias=True)
        load_bulk()

        bg = {}

        def bg_add(u, fn, *args):
            bg.setdefault(u, []).append((fn, args))

        # V for keys chunk si: needed by consume of unit j=si//2 (iter +LAG).
        bg_add(0, v_proj, 0)
        bg_add(0, v_proj, 1)
        for k in range(1, NJ):
            bg_add(k, v_proj, 2 * k)
            bg_add(k, v_proj, 2 * k + 1)
        # remaining K0/Q0 column blocks: scores at iteration j read K cols
        # kc=2j,2j+1 (c-block j//2); Q c-block c is read from iteration 16c.
        for c in (1, 2, 3):
            bg_add(2 * c - 1, kq_col, wkv, dwkv, ktr, 0, c, False)
            bg_add(2 * c, kq_col, wqv, dwqv, qtr, 0, c, True)
        bg_add(1, load_bulk2)
        bg_add(8, load_wrest)
        bg_add(24, load_wo)
        # K/Q projections for m-tile mi: spread over window mi-1, merged DMA.
        for mi in range(1, NMT):
            base = 64 * (mi - 1)
            for c in range(4):
                bg_add(base + 24 + 8 * c, kq_col_wide, wkv, dwkv, ktr, mi, c, False)
                bg_add(base + 28 + 8 * c, kq_col_wide, wqv, dwqv, qtr, mi, c, True)
        # output projection blocks, spread after each qb's last otp write
        for qb in range(NQB - 1):
            for ec in range(8):
                bg_add(214 + 16 * qb + 2 * ec, out_proj, qb, ec)

        units = [
            (t, qb, hh, j)
            for t in range(NMT)
            for qb in range(NQB)
            for hh in range(2)
            for j in range(NJ)
        ]
        # consume lag: unit u's attn@V runs after scores of unit u+LAG, so
        # the exp engines get LAG units of PE work to hide their latency.
        LAG = 2
        # Engine-balanced exp split: 72 units on DVE fast-exp (Bresenham-
        # spread so Act/DVE exps overlap instead of serializing), the rest
        # exact Exp on Act. The first window keeps DVE free for the
        # V-projection copies.
        DVE_EXP = set(
            u for u in range(len(units))
            if u >= 16 and (u * 9) // 32 != ((u - 1) * 9) // 32
        )
        pending = []
        deferred = {}
        NU = len(units)
        for u in range(NU + LAG + 2):
            if u < NU:
                t, qb, hh, j = units[u]
                sl = att_scores(t, qb, hh, j, u in DVE_EXP)
                pending.append((t, qb, hh, j, sl))
            for fn, args in bg.pop(u, []):
                fn(*args)
            nrm = deferred.pop(u, None)
            if nrm is not None:
                nrm()
            while pending:
                thr = 0
                if u < NU:
                    # hold each group's first consume two extra iterations so
                    # the single po bank has been read by the previous
                    # group's normalization multiply
                    thr = LAG + (2 if pending[0][3] == 0 else 0)
                if len(pending) <= thr:
                    break
                nrm = att_consume(*pending.pop(0))
                if nrm is not None:
                    if u < NU:
                        deferred[u + 1] = nrm
                    else:
                        # drain: emit immediately so the next group's j0
                        # never precedes this group's normalization in the
                        # PE stream (in-order cycle hazard)
                        nrm()
                if u < NU:
                    break
        # drain: leftover norms, bg tasks scheduled past the end, then the
        # last qb's output projection
        for u2 in sorted(deferred.keys()):
            deferred[u2]()
        deferred.clear()
        for u in sorted(bg.keys()):
            for fn, args in bg[u]:
                fn(*args)
        for ec in range(8):
            out_proj(NQB - 1, ec, pool=(s_ps if ec % 2 else sm_ps))

    return nc


_NC = None


def _get_program():
    global _NC
    if _NC is None:
        _NC = _build_program()
    return _NC


_F8 = ml_dtypes.float8_e4m3


def _pair(a, chunks):
    """[N, M] -> DoubleRow pair layout [128, chunks*2*M], n = 256c+128i+p."""
    n, m = a.shape
    assert n == chunks * 256
    return np.ascontiguousarray(
        a.reshape(chunks, 2, 128, m).transpose(2, 0, 1, 3).reshape(128, -1)
    )


def _q8_with_resid(a):
    """fp8 value + fp8 residual of a contiguous array."""
    a8 = a.astype(_F8)
    da8 = (a - a8.astype(np.float32)).astype(_F8)
    return a8, da8


def kernel(x, Wq, bq, Wk, bk, Wv, bv, Wo, bo):
    bf = ml_dtypes.bfloat16
    x = np.asarray(x, np.float32)
    Wq = np.asarray(Wq, np.float32)
    Wk = np.asarray(Wk, np.float32)
    Wv = np.asarray(Wv, np.float32)
    Wo = np.asarray(Wo, np.float32)
    bq = np.asarray(bq, np.float32)
    bv = np.asarray(bv, np.float32)
    bo = np.asarray(bo, np.float32)

    in_maps = []
    for c in range(8):
        b, g = c // 2, c % 2
        sl = slice(g * GM, (g + 1) * GM)
        x8, dx8 = _q8_with_resid(_pair(x[b].T, NC2))
        wq8, dwq8 = _q8_with_resid(_pair(Wq[sl, :].T * WS, NC2))
        wk8, dwk8 = _q8_with_resid(_pair(Wk[sl, :].T * WS, NC2))
        wv8, dwv8 = _q8_with_resid(_pair(Wv[sl, :].T * WS, NC2))
        wo_slice = Wo[:, sl].T  # [512, 1024]
        in_maps.append(
            {
                "x8": x8, "dx8": dx8,
                "wq8": wq8, "dwq8": dwq8,
                "wk8": wk8, "dwk8": dwk8,
                "wv8": wv8, "dwv8": dwv8,
                "wo": np.ascontiguousarray(
                    wo_slice.reshape(NMT, 128, D).transpose(1, 0, 2).reshape(
                        128, NMT * D
                    )
                ).astype(bf),
                "bq": np.ascontiguousarray(bq[sl] * WS),
            }
        )

    nc = _get_program()
    res = run_bass_kernel_spmd(nc, in_maps, core_ids=list(range(8)))
    # host-side bias: out = attn_hat @ Wo.T + (bo + Wo @ bv); the V bias bv
    # passes through softmax-weighted averaging unchanged (weights sum to 1).
    # The on-core result carries the WS scale from V.
    bo_full = bo + Wo @ bv
    out = np.empty((B, S, D), np.float32)
    for b in range(B):
        acc = res.results[2 * b]["outT"].astype(np.float32) + res.results[
            2 * b + 1
        ]["outT"].astype(np.float32)
        out[b] = acc.T / WS + bo_full[None, :]
    return out


# revision 28
# speedup vs baseline: 1.0119x; 1.0073x over previous
"""Multi-head self-attention (B=4, S=2048, D=1024, H=16, Hd=64) on 8 TRN2 cores.

Sharding: core c -> (batch b = c//2, head-group g = c%2 of 8 heads).
Each core computes its batch's 8 heads end-to-end plus the partial output
projection for its head group; the host sums the two head-group partials
per batch and adds the (bo + Wo@bv) bias once. No collectives.

v5: Q/K/V projections run as fp8e4 DoubleRow matmuls with K=256 per pass
and scaled residual compensation: 32*y = x8@w8 + x8@dw8 + dx8@w8 with
w8 = f8(32*W), dw8 = f8(32*W - w8), dx8 = f8(x - f8(x)). Three passes cost
25% less PE time than bf16's eight K=128 passes while matching bf16
accuracy (~0.14% projection error). The softmax denominator is folded into
the attn@V matmul via a ones-column appended to V (po psum row 64). The
K-projection bias is dropped: a k-independent additive score offset
cancels exactly in softmax. The output projection stays bf16 (attention
outputs are activations; e4m3 storage would cost ~1.5e-2 accuracy). The
exp (33.5M elements/core) is the roofline: split across Activation (exact
Exp) and DVE (Schraudolph fast-exp), the only engines that can read PSUM;
the PE (~60% busy) absorbs pipeline stalls.
"""

from contextlib import ExitStack

import numpy as np
import ml_dtypes

import concourse.bass as bass
import concourse.tile as tile
from concourse import mybir
from concourse.bass_utils import run_bass_kernel_spmd
from concourse.vector_clock import ScopedClock
from bass_rust import InstNoOp, SyncInfo

BF16 = mybir.dt.bfloat16
F32 = mybir.dt.float32
FP8 = mybir.dt.float8e4
AF = mybir.ActivationFunctionType
DR = mybir.MatmulPerfMode.DoubleRow

B, S, D = 4, 2048, 1024
H, HD = 16, 64
GH = 8          # heads per core
GM = GH * HD    # 512 head dims per core
NQB = 4         # q blocks of 512
QB = 512
NKC = 16        # k chunks of 128
NJ = NKC // 2   # k chunk pairs
NC2 = 4         # 256-deep contraction chunks for D=1024
NMT = GM // 128  # 4 m-tiles
VW = 128        # padded stationary width per (j, head) in v8 (64 V + 1 ones)
WS = 32.0       # fp8 weight/residual scale (carried by Q/K/V, undone on host)
ESC = 0.125 / (WS * WS)  # exp scale: scores carry WS^2

_META_TYPES = ("TileBranchInst", "BassTileLoopBlock", "BassTilePoolBoundary")


class _TileCtx(tile.TileContext):
    """Splits multi-sem-wait instructions: the pinned walrus rejects any TPB
    instruction carrying more than one sem-wait, while Tile emits joins and a
    global end-of-context drain with several."""

    def _split_waits(self, ordered):
        nc = self.nc
        for bb_name, insts in ordered.items():
            out = []
            for inst in insts:
                si = inst.sync_info
                if (
                    si is not None
                    and si.on_wait
                    and len(si.on_wait) > 1
                    and type(inst).__name__ not in _META_TYPES
                    and inst.engine != mybir.EngineType.Unassigned
                ):
                    waits = list(si.on_wait)
                    for w in waits[:-1]:
                        nop = InstNoOp(
                            name=nc.get_next_instruction_name(), ins=[], outs=[]
                        )
                        nop.engine = inst.engine
                        nop.sync_info = SyncInfo(on_wait=[w], on_update=[])
                        out.append(nop)
                    inst.sync_info = SyncInfo(
                        on_wait=[waits[-1]], on_update=list(si.on_update)
                    )
                out.append(inst)
            ordered[bb_name] = out

    def _lower_ordered_insts(self, ordered):
        self._split_waits(ordered)
        super()._lower_ordered_insts(ordered)

    def _drain_and_barrier(self, tick_clock, wait_clock):
        drain_inst = self.nc.sync.drain()
        wait_clock.add_sem_waits(
            drain_inst.ins, ScopedClock({None: tick_clock.global_clock})
        )
        si = drain_inst.ins.sync_info
        waits = list(si.on_wait) if si is not None else []
        if len(waits) > 1:
            drain_inst.ins.sync_info = SyncInfo(
                on_wait=waits[:1], on_update=list(si.on_update)
            )
            for w in waits[1:]:
                extra = self.nc.sync.drain()
                extra.ins.sync_info = SyncInfo(on_wait=[w], on_update=[])

        self.nc.all_engine_barrier()
        assert self.sems is not None
        popped = self.nc._tile_sem_poison_stack.pop()
        assert popped is self._sem_poison
        self.nc.clear_and_free_semaphores(list(self.sems.allocated().values()))
        self.nc.all_engine_barrier()


def _build_program():
    nc = bass.Bass(trn_type="TRN2", debug=False, num_devices=8)

    # fp8 operands in the DoubleRow pair layout: n = 256c + 128i + p.
    x8 = nc.dram_tensor("x8", [128, NC2 * 2 * S], FP8, kind="ExternalInput").ap()
    dx8 = nc.dram_tensor("dx8", [128, NC2 * 2 * S], FP8, kind="ExternalInput").ap()
    wq8 = nc.dram_tensor("wq8", [128, NC2 * 2 * GM], FP8, kind="ExternalInput").ap()
    dwq8 = nc.dram_tensor("dwq8", [128, NC2 * 2 * GM], FP8, kind="ExternalInput").ap()
    wk8 = nc.dram_tensor("wk8", [128, NC2 * 2 * GM], FP8, kind="ExternalInput").ap()
    dwk8 = nc.dram_tensor("dwk8", [128, NC2 * 2 * GM], FP8, kind="ExternalInput").ap()
    wv8 = nc.dram_tensor("wv8", [128, NC2 * 2 * GM], FP8, kind="ExternalInput").ap()
    dwv8 = nc.dram_tensor("dwv8", [128, NC2 * 2 * GM], FP8, kind="ExternalInput").ap()
    # bf16 out-projection weights, pair-major: [128, 4 mt x 1024]
    wo = nc.dram_tensor("wo", [128, NMT * D], BF16, kind="ExternalInput").ap()
    bq = nc.dram_tensor("bq", [GM], F32, kind="ExternalInput").ap()
    outT = nc.dram_tensor("outT", [D, S], F32, kind="ExternalOutput").ap()

    with _TileCtx(nc) as tc, ExitStack() as ctx:
        const_pool = ctx.enter_context(tc.tile_pool(name="const", bufs=1))
        act_pool = ctx.enter_context(tc.tile_pool(name="acts", bufs=1))
        qk8_pool = ctx.enter_context(tc.tile_pool(name="qk8", bufs=2))
        slab_pool = ctx.enter_context(tc.tile_pool(name="slab", bufs=8))
        y_pool = ctx.enter_context(tc.tile_pool(name="y", bufs=8))
        s_ps = ctx.enter_context(tc.tile_pool(name="s_ps", bufs=3, space="PSUM"))
        o_ps = ctx.enter_context(tc.tile_pool(name="o_ps", bufs=1, space="PSUM"))
        sm_ps = ctx.enter_context(tc.tile_pool(name="sm_ps", bufs=1, space="PSUM"))

        # ---- weights / inputs / constants --------------------------------
        bq_sb = const_pool.tile([128, NMT], F32, tag="bq")
        nc.sync.dma_start(bq_sb[:], bq.rearrange("(c p) -> p c", p=128))

        wsz = NC2 * 2 * GM
        x_sb = const_pool.tile([128, NC2 * 2 * S], FP8, tag="x8")
        dx_sb = const_pool.tile([128, NC2 * 2 * S], FP8, tag="dx8")
        xv = x_sb[:].rearrange("p (c i s) -> p c i s", i=2, s=S)
        dxv = dx_sb[:].rearrange("p (c i s) -> p c i s", i=2, s=S)
        x8v = x8.rearrange("p (c i s) -> p c i s", i=2, s=S)
        dx8v = dx8.rearrange("p (c i s) -> p c i s", i=2, s=S)
        wk_sb = const_pool.tile([128, wsz], FP8, tag="wk")
        dwk_sb = const_pool.tile([128, wsz], FP8, tag="dwk")
        wq_sb = const_pool.tile([128, wsz], FP8, tag="wq")
        dwq_sb = const_pool.tile([128, wsz], FP8, tag="dwq")
        wkvr = wk_sb[:].rearrange("p (c i m) -> p c i m", i=2, m=GM)
        dwkvr = dwk_sb[:].rearrange("p (c i m) -> p c i m", i=2, m=GM)
        wqvr = wq_sb[:].rearrange("p (c i m) -> p c i m", i=2, m=GM)
        dwqvr = dwq_sb[:].rearrange("p (c i m) -> p c i m", i=2, m=GM)
        wk8v = wk8.rearrange("p (c i m) -> p c i m", i=2, m=GM)
        dwk8v = dwk8.rearrange("p (c i m) -> p c i m", i=2, m=GM)
        wq8v = wq8.rearrange("p (c i m) -> p c i m", i=2, m=GM)
        dwq8v = dwq8.rearrange("p (c i m) -> p c i m", i=2, m=GM)

        # startup-critical loads first: x/dx s-block 0 and the mi=0 weight
        # slices, so the first K/Q projection chains can begin immediately.
        nc.sync.dma_start(xv[:, :, :, 0:QB], x8v[:, :, :, 0:QB])
        nc.sync.dma_start(dxv[:, :, :, 0:QB], dx8v[:, :, :, 0:QB])
        for dst, src in (
            (wkvr, wk8v), (dwkvr, dwk8v), (wqvr, wq8v), (dwqvr, dwq8v),
        ):
            nc.sync.dma_start(dst[:, :, :, 0:128], src[:, :, :, 0:128])
        wv_sb = const_pool.tile([128, wsz], FP8, tag="wv")
        dwv_sb = const_pool.tile([128, wsz], FP8, tag="dwv")

        def load_bulk():
            nc.sync.dma_start(wv_sb[:], wv8[:, :])
            nc.sync.dma_start(dwv_sb[:], dwv8[:, :])
            nc.sync.dma_start(
                xv[:, :, :, QB : 2 * QB], x8v[:, :, :, QB : 2 * QB]
            )
            nc.sync.dma_start(
                dxv[:, :, :, QB : 2 * QB], dx8v[:, :, :, QB : 2 * QB]
            )

        def load_bulk2():
            for sc in range(2, 4):
                nc.sync.dma_start(
                    xv[:, :, :, sc * QB : (sc + 1) * QB],
                    x8v[:, :, :, sc * QB : (sc + 1) * QB],
                )
                nc.sync.dma_start(
                    dxv[:, :, :, sc * QB : (sc + 1) * QB],
                    dx8v[:, :, :, sc * QB : (sc + 1) * QB],
                )

        def load_wrest():
            for dst, src2 in (
                (wkvr, wk8v), (dwkvr, dwk8v), (wqvr, wq8v), (dwqvr, dwq8v),
            ):
                nc.sync.dma_start(dst[:, :, :, 128:GM], src2[:, :, :, 128:GM])
        wo_sb = const_pool.tile([128, NMT * D], BF16, tag="wo")

        def load_wo():
            nc.sync.dma_start(wo_sb[:], wo[:, :])

        wqv = wq_sb[:].rearrange("p (c i m) -> p c i m", i=2, m=GM)
        dwqv = dwq_sb[:].rearrange("p (c i m) -> p c i m", i=2, m=GM)
        wkv = wk_sb[:].rearrange("p (c i m) -> p c i m", i=2, m=GM)
        dwkv = dwk_sb[:].rearrange("p (c i m) -> p c i m", i=2, m=GM)
        wvv = wv_sb[:].rearrange("p (c i m) -> p c i m", i=2, m=GM)
        dwvv = dwv_sb[:].rearrange("p (c i m) -> p c i m", i=2, m=GM)

        ones64 = const_pool.tile([1, 64], BF16, tag="ones64")
        nc.vector.memset(ones64[:], 1.0)
        rd = [
            act_pool.tile([1, QB], BF16, name=f"rd{i}", tag=f"rd{i}")
            for i in range(2)
        ]
        tmp_o = [
            act_pool.tile([64, QB], BF16, name=f"tmpo{i}", tag=f"tmpo{i}")
            for i in range(2)
        ]
        bc_sb = [
            act_pool.tile([64, QB], BF16, name=f"bcs{i}", tag=f"bcs{i}")
            for i in range(2)
        ]

        # ---- persistent activations -------------------------------------
        qtr = act_pool.tile([32, GH * 2 * S], FP8, tag="qtr")
        ktr = act_pool.tile([32, GH * 2 * S], FP8, tag="ktr")
        # v8: [128 keys, 8 jpairs x 8 heads x (2 chunk x 128 pad)] fp8;
        # cols 0:64 = V dims, col 64 = ones (softmax denominator), rest pad.
        v8 = act_pool.tile([128, NJ * GH * 2 * VW], FP8, tag="v8")
        v8v = v8[:].rearrange("p (j h i m) -> p j h i m", h=GH, i=2, m=VW)
        nc.vector.memset(v8v[:, :, :, :, 64:65], 1.0)
        # O^T per head-pair: [128, S] bf16; even head rows 0-63, odd 64-127.
        otp = [
            act_pool.tile([128, S], BF16, name=f"otp{t}", tag=f"otp{t}")
            for t in range(NMT)
        ]

        # Warm the PE p-state so the first projection chains run at full
        # clock: a stream of tiny matmuls on constant data.
        wps = sm_ps.tile([64, 64], F32, name="wps", tag="sm")
        for i in range(56):
            nc.tensor.matmul(
                wps[:], ones64[:], ones64[:], start=True, stop=True,
            )

        # ---- background task closures ------------------------------------
        def kq_chain(wv_, dwv_, mi, c):
            """12 fp8-DR passes: x8@w8 + x8@dw8 + dx8@w8, K=256 each."""
            ps = sm_ps.tile([128, QB], F32, name="kqp", tag="sm")
            msl = slice(mi * 128, (mi + 1) * 128)
            csl = slice(c * QB, (c + 1) * QB)
            n = 0
            for dc in range(NC2):
                for lhs, rhs in (
                    (wv_[:, dc, :, msl], xv[:, dc, :, csl]),
                    (dwv_[:, dc, :, msl], xv[:, dc, :, csl]),
                    (wv_[:, dc, :, msl], dxv[:, dc, :, csl]),
                ):
                    n += 1
                    nc.tensor.matmul(
                        ps[:], lhs, rhs,
                        start=(n == 1),
                        stop=(n == 3 * NC2),
                        perf_mode=DR,
                    )
            return ps

        def kq_dma(dst, q8, mi, c0, w):
            """Rearrange q8 [128, w] into the pair-split [32, h, i, s] layout
            of qtr/ktr; 4 plain-partition-range DMAs."""
            dv = dst[:].rearrange("p (h i s) -> p h i s", i=2, s=S)
            for hh in range(2):
                for i in range(2):
                    h = 2 * mi + hh
                    nc.sync.dma_start(
                        dv[:, h, i, c0 * QB : c0 * QB + w],
                        q8[hh * 64 + i * 32 : hh * 64 + i * 32 + 32, 0:w],
                    )

        def kq_col(wv_, dwv_, dst, mi, c, bias):
            """mi=0 path: single 512-col block + immediate rearrange DMAs."""
            ps = kq_chain(wv_, dwv_, mi, c)
            q8 = qk8_pool.tile([128, QB], FP8, name="q8blk", tag="q8")
            with tc.high_priority(offset=150):
                if bias:
                    nc.vector.tensor_scalar_add(
                        q8[:], ps[:], bq_sb[:, mi : mi + 1]
                    )
                else:
                    nc.vector.tensor_copy(q8[:], ps[:])
            kq_dma(dst, q8, mi, c, QB)

        kq_full = {}

        def kq_col_wide(wv_, dwv_, dst, mi, c, bias):
            """mi>=1 path: accumulate 4 col-blocks into one [128, 2048] tile,
            then 4 merged DMAs when the last block lands."""
            key = (dst is qtr, mi)
            if c == 0:
                kq_full[key] = qk8_pool.tile(
                    [128, S], FP8, name="q8full", tag="q8f"
                )
            q8 = kq_full[key]
            ps = kq_chain(wv_, dwv_, mi, c)
            with tc.high_priority(offset=150):
                if bias:
                    nc.vector.tensor_scalar_add(
                        q8[:, c * QB : (c + 1) * QB], ps[:],
                        bq_sb[:, mi : mi + 1],
                    )
                else:
                    nc.vector.tensor_copy(q8[:, c * QB : (c + 1) * QB], ps[:])
            if c == NC2 - 1:
                kq_dma(dst, q8, mi, 0, S)

        def v_proj(si):
            """V projection for keys chunk si, all 8 heads at once."""
            j, par = si // 2, si % 2
            pool = sm_ps if si % 3 == 0 else s_ps
            ps = pool.tile([128, GM], F32, name="vp",
                           tag="sm" if si % 3 == 0 else "s2")
            csl = slice(si * 128, (si + 1) * 128)
            n = 0
            for dc in range(NC2):
                for lhs, rhs in (
                    (xv[:, dc, :, csl], wvv[:, dc, :, :]),
                    (xv[:, dc, :, csl], dwvv[:, dc, :, :]),
                    (dxv[:, dc, :, csl], wvv[:, dc, :, :]),
                ):
                    n += 1
                    nc.tensor.matmul(
                        ps[:], lhs, rhs,
                        start=(n == 1),
                        stop=(n == 3 * NC2),
                        perf_mode=DR,
                    )
            with tc.high_priority(offset=150):
                nc.vector.tensor_copy(
                    v8v[:, j : j + 1, :, par : par + 1, 0:64],
                    ps[:].rearrange("p (h m) -> p h m", h=GH),
                )

        def out_proj(qb, ec, pool=None):
            pool = pool or sm_ps
            ps = pool.tile([128, QB], F32, name="yp",
                           tag="sm" if pool is sm_ps else ("po" if pool is o_ps else "s2"))
            for mt in range(NMT):
                nc.tensor.matmul(
                    ps[:],
                    wo_sb[:, mt * D + ec * 128 : mt * D + (ec + 1) * 128],
                    otp[mt][:, qb * QB : (qb + 1) * QB],
                    start=(mt == 0),
                    stop=(mt == NMT - 1),
                )
            y_sb = y_pool.tile([128, QB], F32, name="yblk", tag="y")
            with tc.high_priority(offset=150):
                nc.vector.tensor_copy(y_sb[:], ps[:])
            nc.sync.dma_start(
                outT[ec * 128 : (ec + 1) * 128, qb * QB : (qb + 1) * QB], y_sb[:]
            )

        # ---- attention unit emitters -------------------------------------
        # DVE fast-exp: e4m3 bits of exp(s*ESC) are an affine function of s
        # (linear-mantissa log2 trick): bits = s*ESC*8/ln2 + 56 + c.
        FEXP_A = ESC * 8.0 / float(np.log(2.0))
        FEXP_B = 55.55  # HW convert rounds-to-nearest; Schraudolph-centered

        def att_scores(t, qb, hh, j, on_dve):
            """Scores + exp for head 2t+hh at (qb, kc-pair j)."""
            h = 2 * t + hh
            with tc.high_priority(offset=300):
                s2 = s_ps.tile([128, 2 * QB], F32, name="s2", tag="s2")
                qv = qtr[:, h * 2 * S : (h + 1) * 2 * S].rearrange(
                    "p (i s) -> p i s", i=2
                )[:, :, qb * QB : (qb + 1) * QB]
                for par in range(2):
                    kc = 2 * j + par
                    kv = ktr[:, h * 2 * S : (h + 1) * 2 * S].rearrange(
                        "p (i s) -> p i s", i=2
                    )[:, :, kc * 128 : (kc + 1) * 128]
                    nc.tensor.matmul(
                        s2[:, par * QB : (par + 1) * QB],
                        kv, qv, start=True, stop=True, perf_mode=DR,
                    )
                sl = slab_pool.tile([128, 2 * QB], FP8, name="sl", tag="slab")
                if on_dve:
                    nc.vector.tensor_scalar(
                        sl[:].bitcast(mybir.dt.uint8),
                        s2[:],
                        FEXP_A,
                        FEXP_B,
                        mybir.AluOpType.mult,
                        mybir.AluOpType.add,
                    )
                else:
                    nc.scalar.activation(sl[:], s2[:], AF.Exp, scale=ESC)
            return sl

        state = {"po": None, "n": 0}

        def att_consume(t, qb, hh, j, sl):
            """attn@V (+ folded denominator) for unit (t, qb, hh, j).
            Returns a deferred normalization closure at group end so the PE's
            bc matmul is emitted well after the reciprocal it waits on."""
            h = 2 * t + hh
            if j == 0:
                state["po"] = o_ps.tile([65, QB], F32, name="po", tag="po")
            po = state["po"]
            rhs = sl[:].rearrange("p (i q) -> p i q", i=2)
            lhsT = v8v[:, j, h, :, 0:65]
            nc.tensor.matmul(
                po[:, :], lhsT, rhs,
                start=(j == 0), stop=(j == NJ - 1), perf_mode=DR,
            )
            if j != NJ - 1:
                return None
            k = state["n"] % 2
            state["n"] += 1
            r = rd[k]
            with tc.high_priority(offset=150):
                with nc.allow_low_precision(reason="softmax recip in bf16"):
                    nc.vector.reciprocal(r[0:1, :], po[64:65, :])

            def norm():
                bc = s_ps.tile([64, QB], F32, name="bc", tag="s2")
                nc.tensor.matmul(bc[:], ones64[:], r[:], start=True, stop=True)
                bcs = bc_sb[k]
                nc.vector.tensor_copy(bcs[:], bc[:])
                if hh == 0:
                    nc.vector.tensor_mul(
                        otp[t][0:64, qb * QB : (qb + 1) * QB], po[0:64, :], bcs[:]
                    )
                else:
                    tm = tmp_o[k]
                    nc.vector.tensor_mul(tm[:], po[0:64, :], bcs[:])
                    nc.sync.dma_start(
                        otp[t][64:128, qb * QB : (qb + 1) * QB], tm[:]
                    )

            return norm

        # ---- the flat schedule -------------------------------------------
        kq_col(wkv, dwkv, ktr, 0, 0, bias=False)
        kq_col(wqv, dwqv, qtr, 0, 0, bias=True)
        load_bulk()

        bg = {}

        def bg_add(u, fn, *args):
            bg.setdefault(u, []).append((fn, args))

        # V for keys chunk si: needed by consume of unit j=si//2 (iter +LAG).
        bg_add(0, v_proj, 0)
        bg_add(0, v_proj, 1)
        for k in range(1, NJ):
            bg_add(k, v_proj, 2 * k)
            bg_add(k, v_proj, 2 * k + 1)
        # remaining K0/Q0 column blocks: scores at iteration j read K cols
        # kc=2j,2j+1 (c-block j//2); Q c-block c is read from iteration 16c.
        for c in (1, 2, 3):
            bg_add(2 * c - 1, kq_col, wkv, dwkv, ktr, 0, c, False)
            bg_add(2 * c, kq_col, wqv, dwqv, qtr, 0, c, True)
        bg_add(1, load_bulk2)
        bg_add(8, load_wrest)
        bg_add(24, load_wo)
        # K/Q projections for m-tile mi: spread over window mi-1, merged DMA.
        for mi in range(1, NMT):
            base = 64 * (mi - 1)
            for c in range(4):
                bg_add(base + 24 + 8 * c, kq_col_wide, wkv, dwkv, ktr, mi, c, False)
                bg_add(base + 28 + 8 * c, kq_col_wide, wqv, dwqv, qtr, mi, c, True)
        # output projection blocks, spread after each qb's last otp write
        for qb in range(NQB - 1):
            for ec in range(8):
                bg_add(214 + 16 * qb + 2 * ec, out_proj, qb, ec)

        units = [
            (t, qb, hh, j)
            for t in range(NMT)
            for qb in range(NQB)
            for hh in range(2)
            for j in range(NJ)
        ]
        # consume lag: unit u's attn@V runs after scores of unit u+LAG, so
        # the exp engines get LAG units of PE work to hide their latency.
        LAG = 2
        # Engine-balanced exp split: 72 units on DVE fast-exp (Bresenham-
        # spread so Act/DVE exps overlap instead of serializing), the rest
        # exact Exp on Act. The first window keeps DVE free for the
        # V-projection copies.
        DVE_EXP = set(
            u for u in range(len(units))
            if u >= 16 and (u * 9) // 32 != ((u - 1) * 9) // 32
        )
        pending = []
        deferred = {}
        NU = len(units)
        for u in range(NU + LAG + 2):
            if u < NU:
                t, qb, hh, j = units[u]
                sl = att_scores(t, qb, hh, j, u in DVE_EXP)
                pending.append((t, qb, hh, j, sl))
            for fn, args in bg.pop(u, []):
                fn(*args)
            nrm = deferred.pop(u, None)
            if nrm is not None:
                nrm()
            while pending:
                thr = 0
                if u < NU:
                    # hold each group's first consume two extra iterations so
                    # the single po bank has been read by the previous
                    # group's normalization multiply
                    thr = LAG + (2 if pending[0][3] == 0 else 0)
                if len(pending) <= thr:
                    break
                nrm = att_consume(*pending.pop(0))
                if nrm is not None:
                    if u < NU:
                        deferred[u + 1] = nrm
                    else:
                        # drain: emit immediately so the next group's j0
                        # never precedes this group's normalization in the
                        # PE stream (in-order cycle hazard)
                        nrm()
                if u < NU:
                    break
        # drain: leftover norms, bg tasks scheduled past the end, then the
        # last qb's output projection
        for u2 in sorted(deferred.keys()):
            deferred[u2]()
        deferred.clear()
        for u in sorted(bg.keys()):
            for fn, args in bg[u]:
                fn(*args)
        for ec in range(8):
            out_proj(NQB - 1, ec, pool=s_ps)

    return nc


_NC = None


def _get_program():
    global _NC
    if _NC is None:
        _NC = _build_program()
    return _NC


_F8 = ml_dtypes.float8_e4m3


def _pair(a, chunks):
    """[N, M] -> DoubleRow pair layout [128, chunks*2*M], n = 256c+128i+p."""
    n, m = a.shape
    assert n == chunks * 256
    return np.ascontiguousarray(
        a.reshape(chunks, 2, 128, m).transpose(2, 0, 1, 3).reshape(128, -1)
    )


def _q8_with_resid(a):
    """fp8 value + fp8 residual of a contiguous array."""
    a8 = a.astype(_F8)
    da8 = (a - a8.astype(np.float32)).astype(_F8)
    return a8, da8


def kernel(x, Wq, bq, Wk, bk, Wv, bv, Wo, bo):
    bf = ml_dtypes.bfloat16
    x = np.asarray(x, np.float32)
    Wq = np.asarray(Wq, np.float32)
    Wk = np.asarray(Wk, np.float32)
    Wv = np.asarray(Wv, np.float32)
    Wo = np.asarray(Wo, np.float32)
    bq = np.asarray(bq, np.float32)
    bv = np.asarray(bv, np.float32)
    bo = np.asarray(bo, np.float32)

    in_maps = []
    for c in range(8):
        b, g = c // 2, c % 2
        sl = slice(g * GM, (g + 1) * GM)
        x8, dx8 = _q8_with_resid(_pair(x[b].T, NC2))
        wq8, dwq8 = _q8_with_resid(_pair(Wq[sl, :].T * WS, NC2))
        wk8, dwk8 = _q8_with_resid(_pair(Wk[sl, :].T * WS, NC2))
        wv8, dwv8 = _q8_with_resid(_pair(Wv[sl, :].T * WS, NC2))
        wo_slice = Wo[:, sl].T  # [512, 1024]
        in_maps.append(
            {
                "x8": x8, "dx8": dx8,
                "wq8": wq8, "dwq8": dwq8,
                "wk8": wk8, "dwk8": dwk8,
                "wv8": wv8, "dwv8": dwv8,
                "wo": np.ascontiguousarray(
                    wo_slice.reshape(NMT, 128, D).transpose(1, 0, 2).reshape(
                        128, NMT * D
                    )
                ).astype(bf),
                "bq": np.ascontiguousarray(bq[sl] * WS),
            }
        )

    nc = _get_program()
    res = run_bass_kernel_spmd(nc, in_maps, core_ids=list(range(8)))
    # host-side bias: out = attn_hat @ Wo.T + (bo + Wo @ bv); the V bias bv
    # passes through softmax-weighted averaging unchanged (weights sum to 1).
    # The on-core result carries the WS scale from V.
    bo_full = bo + Wo @ bv
    out = np.empty((B, S, D), np.float32)
    for b in range(B):
        acc = res.results[2 * b]["outT"].astype(np.float32) + res.results[
            2 * b + 1
        ]["outT"].astype(np.float32)
        out[b] = acc.T / WS + bo_full[None, :]
    return out
